# revision 24
# baseline (speedup 1.0000x reference)
"""Trainium2 Bass kernel for nn_DiffNet (gnn_message_passing).

The reference's per-element "edge MLP" over the meta stack (vi, W, vj)
collapses algebraically.  With g = conv1_w.T @ conv2_w[0] (3 scalars),
hb = conv1_b@conv2_w[0]+conv2_b[0], z = vi @ W.T (no bias),
s1[b] = sum_i vi[b,i], s2[b] = sum_i vi[b,i]^2:

    out[b,o] = relu(z+b)[b,o] * (1 + scale*g2*s1[b])
             + scale*(g0*s2[b] + g1*z[b,o] + hb*s1[b])

so the whole network is 3 matmuls + per-batch stats + elementwise.

Implementation notes (fp16 end-to-end on the DMA/matmul dataflow; the
2e-2 rel-err gate leaves ~10x headroom, measured ~2.6e-3):

  * z is computed TRANSPOSED via weight-stationary matmuls: lhsT =
    [128,128] weight block (fast weight load), rhs = a-chunk [128,B].
    (LDW,MM) pairs pipeline at ~50ns, and zt lands [out-feature, batch]
    in PSUM directly -- no z->SBUF copy, no eye transposes.
  * layer bias accumulates into the same psum group as one rank-1
    matmul (bias_row[1,O] stationary x ones[1,B] moving); the spurious
    k1*bias term this adds to the k1*z part of the combine is ~5e-5
    relative -- far below the gate -- so no correction is applied.
  * per-batch stats: column sums of a and a^2 via matmul chains against
    ones-column stationaries, accumulated chunk-wise in PSUM, so s1
    lands on partition 0 and s2 on partition 32 (where the alpha/beta
    coefficient matmuls need them).
  * alpha/beta: rank-1 matmuls with memset-built [96,128] coefficient
    stationaries -> [*,2B] psum, one small SBUF copy; the wide combine
    reads them through stride-0 broadcast APs:
        m = max(zt,0)*alpha ; t = k1*zt + beta ; a' = m + t  (3 DVE ops)
  * the PE's HAM clock gate defaults to 4/8 throttle (1.2 GHz) and only
    reaches 8/8 after ~4us of sustained matmul activity, so a block of
    junk matmuls warms the array while the weight DMA streams.

Distribution (8 cores, no collectives): fc1/fc2 replicated, fc3 sharded
over its output dim (32 cols/core); full batch B=32 on every core; host
concatenates the 8 [32,32] output shards.
"""

import sys

if "/opt/trn_rl_repo" not in sys.path:
    sys.path.insert(0, "/opt/trn_rl_repo")

import numpy as np


def _install_ntff_hook_shim():
    """This image's antenv lacks ``axon_hooks``; bass_utils hard-imports it
    when tracing under axon.  Provide the module and register the ctypes
    NTFF hook from trn_agent_boot so ``trace=True`` yields exec_time_ns."""
    import types

    if "antenv.axon_hooks" in sys.modules:
        return
    try:
        import antenv

        mod = types.ModuleType("antenv.axon_hooks")
        _h = [None]
        mod.set_axon_ntff_profile_hook = lambda hook: _h.__setitem__(0, hook)
        mod.get_axon_ntff_profile_hook = lambda: _h[0]
        sys.modules["antenv.axon_hooks"] = mod
        antenv.axon_hooks = mod
        from trn_agent_boot.trn_boot import _ntff_profile_via_ctypes

        mod.set_axon_ntff_profile_hook(
            _ntff_profile_via_ctypes("/opt/axon/libaxon_pjrt.so")
        )
    except Exception:
        pass


_install_ntff_hook_shim()

N_CORES = 8
B = 32
I1, O1, O2, O3 = 1024, 512, 512, 256
O3L = O3 // N_CORES  # fc3 output cols per core
RATE = 0.1
N_JUNK = 32  # HAM warmup matmuls (64-col) during the DMA stream

_CACHE = {}
LAST_RESULTS = None  # BassKernelResults of the most recent run (for test.py)


def _build(k0, k1, k2, kb):
    import concourse.bacc as bacc
    import concourse.mybir as mybir
    import concourse.tile as tile
    import concourse.bass as bass

    f16 = mybir.dt.float16
    f32 = mybir.dt.float32
    AF = mybir.ActivationFunctionType
    ALU = mybir.AluOpType

    nc = bacc.Bacc(
        "TRN2", target_bir_lowering=False, debug=False, num_devices=N_CORES
    )

    # DRAM parameters (all fp16).  Weights are packed as [128,128] blocks,
    # block order c-outer / k-inner, so each output chunk's accumulation
    # chain is contiguous in the stream.
    # xw1a: cols 0:256 = x.T packed; 256:2304 = w1 blocks for c=0,1
    xw1a = nc.declare_dram_parameter("xw1a", [128, 2304], f16, isOutput=False)
    w1b = nc.declare_dram_parameter("w1b", [128, 2048], f16, isOutput=False)
    # w23: cols 0:2048 = w2 blocks (c0..c3), 2048:2176 = w3 blocks (k0..k3)
    w23 = nc.declare_dram_parameter("w23", [128, 2176], f16, isOutput=False)
    # biasT: per-layer bias columns for the combine's broadcast add
    # (cols 0:4 = fc1_b chunks, 4:8 = fc2_b, 8 = fc3_b[core] on rows 0:32)
    biasT = nc.declare_dram_parameter("biasT", [128, 9], f16, isOutput=False)
    # nk1b: -k1 * (fc1_b | fc2_b | fc3_b[core]) -> row 64 of the beta
    # stationaries, folding the bias correction into the beta matmuls
    nk1b = nc.declare_dram_parameter("nk1b", [1, O1 + O2 + O3L], f16, isOutput=False)
    out_d = nc.declare_dram_parameter("out", [O3L, B], f16, isOutput=True)

    def rep(ap, n):
        """Insert a stride-0 dim of size n before the innermost free dim."""
        return ap.unsqueeze(1).broadcast_to([ap.shape[0], n, ap.shape[1]])

    with tile.TileContext(nc) as tc:
        with (
            tc.tile_pool(name="sb", bufs=1) as sp,
            tc.tile_pool(name="ps", bufs=1, space=bass.MemorySpace.PSUM) as pp,
        ):
            # ---- SBUF tiles
            txw1a = sp.tile([128, 2304], f16, tag="xw1a")
            tx = txw1a[:, 0:256]
            tw1a = txw1a[:, 256:2304]
            tw1b = sp.tile([128, 2048], f16, tag="w1b")
            tw23 = sp.tile([128, 2176], f16, tag="w23")
            tbiasT = sp.tile([128, 9], f16, tag="biasT")
            txsq = sp.tile([128, 256], f16, tag="xsq")
            tka = sp.tile([96, 128], f16, tag="ka")   # alpha: k2@r0, 1@r64
            # beta stationaries: kb@r0, k0@r32, -k1*bias@r64 (per out col)
            tkb = sp.tile([96, O1 + O2 + O3L], f16, tag="kb")
            tones2 = sp.tile([128, 33], f16, tag="ones2")  # cols 0,32 = 1
            tones1b = sp.tile([1, B], f16, tag="ones1b")
            s_sb = [
                sp.tile([96, B], f16, tag=f"ssb{l}", name=f"ssb{l}")
                for l in range(3)
            ]
            tm = [
                sp.tile([128, 4 * B], f16, tag="m1", name="m1"),
                sp.tile([128, 4 * B], f16, tag="m2", name="m2"),
                sp.tile([O3L, B], f16, tag="m3", name="m3"),
            ]
            tt = [
                sp.tile([128, 4 * B], f16, tag="t1", name="t1"),
                sp.tile([128, 4 * B], f16, tag="t2", name="t2"),
                sp.tile([O3L, B], f16, tag="t3", name="t3"),
            ]
            ta2 = sp.tile([128, 4 * B], f16, tag="a2")
            ta2sq = sp.tile([128, 4 * B], f16, tag="a2sq")
            ta3 = sp.tile([128, 4 * B], f16, tag="a3")
            ta3sq = sp.tile([128, 4 * B], f16, tag="a3sq")
            tu = [
                sp.tile([128, 4 * B], f16, tag="u1", name="u1"),
                sp.tile([128, 4 * B], f16, tag="u2", name="u2"),
                sp.tile([O3L, B], f16, tag="u3", name="u3"),
            ]
            out_sb = sp.tile([O3L, B], f16, tag="osb")

            # ---- memsets (gpsimd; ordered before dependent reads)
            g = nc.gpsimd
            g.memset(tka[:], 0.0)
            g.memset(tka[0:1, :], k2)
            g.memset(tka[64:65, :], 1.0)
            g.memset(tkb[:], 0.0)
            g.memset(tkb[0:1, :], kb)
            g.memset(tkb[32:33, :], k0)
            g.memset(tones2[:], 0.0)
            g.memset(tones2[:, 0:1], 1.0)
            g.memset(tones2[:, 32:33], 1.0)
            g.memset(tones1b[:], 1.0)
            for l in range(3):
                g.memset(s_sb[l][:], 1.0)  # junk rows finite; row 64 = ones

            # ---- DMAs.  sync ring: payload in need-order; scalar ring:
            # the tiny bias tensors (nk1b lands in tkb row 64 after the
            # zero/row memsets -- Tile orders the overlap).
            nc.sync.dma_start(txw1a[:], xw1a[:])
            nc.sync.dma_start(tw1b[:], w1b[:])
            nc.sync.dma_start(tw23[:], w23[:])
            nc.scalar.dma_start(tbiasT[:], biasT[:])
            nc.scalar.dma_start(tkb[64:65, :], nk1b[:])

            # PSUM is bank-granular (2KB/partition per tile): pack logical
            # regions into shared bank tiles, grouped by phase.
            # Layout per layer bank: zt [*,C*B] | alpha [*,B] | beta' [*,C*B]
            bankA = pp.tile([128, 512], f32, tag="bkA", name="bankA")
            bankB = pp.tile([128, 512], f32, tag="bkB", name="bankB")
            bankC = pp.tile([O3L, 512], f32, tag="bkC", name="bankC")
            bankS = pp.tile([33, 512], f32, tag="bkS", name="bankS")   # stats
            bankJ = pp.tile([1, 512], f32, tag="bkJ", name="bankJ")    # junk
            zt = [
                bankA[:, 0:4 * B],
                bankB[:, 0:4 * B],
                bankC[:, 0:B],
            ]
            alp = [
                bankA[:, 4 * B:5 * B],
                bankB[:, 4 * B:5 * B],
                bankC[:, B:2 * B],
            ]
            bet = [
                bankA[:, 5 * B:9 * B],
                bankB[:, 5 * B:9 * B],
                bankC[:, 2 * B:3 * B],
            ]
            s1p = [bankS[0:1, l * 2 * B:l * 2 * B + B] for l in range(3)]
            s2p = [bankS[0:33, l * 2 * B + B:(l + 1) * 2 * B] for l in range(3)]

            MM = nc.tensor.matmul

            # ---- HAM warmup: junk matmuls keep the PE array busy while
            # the weight stream lands, releasing the 4/8 clock throttle
            # before the real z chains run.  tka/tkb are memset-built and
            # ready within ~0.5us; results are never read.
            jmov = rep(tkb[0:96, 0:32], 2)  # [96, 2, 32] -> 64 cols
            for _ in range(N_JUNK):
                MM(bankJ[0:1, 0:64], tka[0:96, 0:1], jmov, start=True, stop=True)

            def stats(l, a_t, asq_t, C):
                """column sums of a (->s1p, partition 0) and a^2 (->s2p,
                partition 32), chunk-accumulated in psum."""
                for c in range(C):
                    MM(s1p[l][:], tones2[:, 0:1], a_t[:, c * B:(c + 1) * B],
                       start=(c == 0), stop=(c == C - 1))
                for c in range(C):
                    MM(s2p[l][:], tones2[:, 0:33], asq_t[:, c * B:(c + 1) * B],
                       start=(c == 0), stop=(c == C - 1))

            def stats_copies(l):
                nc.vector.tensor_copy(s_sb[l][0:1, 0:B], s1p[l][:])
                nc.vector.tensor_copy(s_sb[l][32:33, 0:B], s2p[l][32:33, 0:B])

            def ab_mms(l, C, ocols, boff):
                """alpha[b] broadcast across partitions; beta'[o,b] =
                kb*s1 + k0*s2 - k1*bias[o] via per-chunk stationaries."""
                MM(alp[l][0:ocols, 0:B], tka[:, 0:ocols], s_sb[l][0:96, 0:B],
                   start=True, stop=True)
                for c in range(C):
                    MM(bet[l][0:ocols, c * B:(c + 1) * B],
                       tkb[:, boff + c * ocols:boff + (c + 1) * ocols],
                       s_sb[l][0:96, 0:B], start=True, stop=True)

            def zt_chains(l, a_t, w_t, K, C, ocols):
                """zt[l] = (a.T @ w).T (biasless) via weight-stationary
                [128,ocols] blocks, K (LDW,MM) pairs per out-chunk."""
                for c in range(C):
                    dst = zt[l][0:ocols, c * B:(c + 1) * B]
                    for k in range(K):
                        MM(dst, w_t[:, (c * K + k) * ocols:(c * K + k + 1) * ocols],
                           a_t[:, k * B:(k + 1) * B],
                           start=(k == 0), stop=(k == K - 1))

            def bias_bc(l, C, np_out, bcol):
                """biasT column(s) broadcast over batch: [np, C] -> [np, C, B]
                with a stride-0 innermost dim."""
                ap = tbiasT[0:np_out, bcol:bcol + C]
                return ap.unsqueeze(2).broadcast_to([np_out, C, B])

            def combine(l, C, np_out, bcol, a_out, sq_out):
                """u = zt + bias ; a_out = max(u,0)*alpha + (k1*u + beta')
                with alpha/beta' read straight from PSUM (one psum src per
                op) and bias via a stride-0 broadcast of biasT."""
                n = C * B
                ztv = zt[l][0:np_out, 0:n]
                uv = tu[l][0:np_out, 0:n]
                nc.vector.tensor_tensor(uv, ztv, bias_bc(l, C, np_out, bcol),
                                        ALU.add)
                al = alp[l][0:np_out, 0:B]
                if C > 1:
                    al = rep(al, C)
                nc.vector.scalar_tensor_tensor(
                    tm[l][0:np_out, 0:n], uv, 0.0, al, ALU.max, ALU.mult)
                nc.vector.scalar_tensor_tensor(
                    tt[l][0:np_out, 0:n], uv, k1, bet[l][0:np_out, 0:n],
                    ALU.mult, ALU.add)
                nc.vector.tensor_tensor(
                    a_out[0:np_out, 0:n], tm[l][0:np_out, 0:n],
                    tt[l][0:np_out, 0:n], ALU.add)
                if sq_out is not None:
                    nc.scalar.activation(
                        sq_out[0:np_out, 0:n], a_out[0:np_out, 0:n], AF.Square)

            # ================= layer 1 =================
            nc.vector.tensor_tensor(txsq[:], tx, tx, ALU.mult)
            stats(0, tx, txsq[:], 8)
            stats_copies(0)
            # c=0,1 blocks stream in xw1a; c=2,3 in w1b
            for c in range(2):
                for k in range(8):
                    MM(zt[0][:, c * B:(c + 1) * B],
                       tw1a[:, (c * 8 + k) * 128:(c * 8 + k + 1) * 128],
                       tx[:, k * B:(k + 1) * B], start=(k == 0), stop=(k == 7))
            for c in range(2):
                for k in range(8):
                    MM(zt[0][:, (2 + c) * B:(3 + c) * B],
                       tw1b[:, (c * 8 + k) * 128:(c * 8 + k + 1) * 128],
                       tx[:, k * B:(k + 1) * B], start=(k == 0), stop=(k == 7))
            ab_mms(0, 4, 128, 0)
            combine(0, 4, 128, 0, ta2[:], ta2sq[:])

            # ================= layer 2 =================
            stats(1, ta2[:], ta2sq[:], 4)
            zt_chains(1, ta2[:], tw23[:, 0:2048], 4, 4, 128)
            stats_copies(1)
            ab_mms(1, 4, 128, O1)
            combine(1, 4, 128, 4, ta3[:], ta3sq[:])

            # ================= layer 3 =================
            stats(2, ta3[:], ta3sq[:], 4)
            zt_chains(2, ta3[:], tw23[:, 2048:2176], 4, 1, O3L)
            stats_copies(2)
            ab_mms(2, 1, O3L, O1 + O2)
            combine(2, 1, O3L, 8, out_sb[:], None)

            nc.sync.dma_start(out_d[:], out_sb[:])

    nc.compile()
    return nc


def kernel(**inputs):
    from concourse.bass_utils import run_bass_kernel_spmd

    x = np.asarray(inputs["x"], dtype=np.float32)
    fc1_w = np.asarray(inputs["fc1_w"], dtype=np.float32)
    fc1_b = np.asarray(inputs["fc1_b"], dtype=np.float32)
    fc2_w = np.asarray(inputs["fc2_w"], dtype=np.float32)
    fc2_b = np.asarray(inputs["fc2_b"], dtype=np.float32)
    fc3_w = np.asarray(inputs["fc3_w"], dtype=np.float32)
    fc3_b = np.asarray(inputs["fc3_b"], dtype=np.float32)
    c1w = np.asarray(inputs["conv1_w"], dtype=np.float32)
    c1b = np.asarray(inputs["conv1_b"], dtype=np.float32)
    c2w = np.asarray(inputs["conv2_w"], dtype=np.float32)
    c2b = np.asarray(inputs["conv2_b"], dtype=np.float32)
    bn = float(np.asarray(inputs["batch_num"]).astype(np.float64))

    scale = np.float32(RATE) / np.float32(bn)
    gv = (c1w.T @ c2w[0]).astype(np.float32)  # [3]
    hb = np.float32(c1b @ c2w[0] + c2b[0])
    k0 = float(scale * gv[0])
    k1 = float(scale * gv[1])
    k2 = float(scale * gv[2])
    kb = float(scale * hb)

    key = (k0, k1, k2, kb)
    if key not in _CACHE:
        _CACHE[key] = _build(*key)
    nc = _CACHE[key]

    def pack_x(m):  # [1024, 32] -> [128, 8*32]
        return np.ascontiguousarray(
            m.reshape(8, 128, B).transpose(1, 0, 2).reshape(128, 8 * B)
        ).astype(np.float16)

    def pack_blocks(wt, K, C, ocols):
        """wt [K*128, C*ocols] -> [128, C*K*ocols], block order c-outer
        k-inner: block (c,k) = wt[k*128:(k+1)*128, c*ocols:(c+1)*ocols]."""
        out = np.empty((128, C * K * ocols), dtype=np.float16)
        for c in range(C):
            for k in range(K):
                out[:, (c * K + k) * ocols:(c * K + k + 1) * ocols] = wt[
                    k * 128:(k + 1) * 128, c * ocols:(c + 1) * ocols
                ]
        return out

    w1_h = pack_blocks(fc1_w.T, 8, 4, 128)  # [128, 4096]
    xw1a_h = np.zeros((128, 2304), dtype=np.float16)
    xw1a_h[:, 0:256] = pack_x(x.T)
    xw1a_h[:, 256:2304] = w1_h[:, 0:2048]
    w1b_h = np.ascontiguousarray(w1_h[:, 2048:4096])
    w2_h = pack_blocks(fc2_w.T, 4, 4, 128)  # [128, 2048]

    in_maps = []
    for c in range(N_CORES):
        sl = slice(c * O3L, (c + 1) * O3L)
        w3_h = pack_blocks(fc3_w[sl].T, 4, 1, O3L)  # [128, 128]
        w23_h = np.concatenate([w2_h, w3_h], axis=1)
        bias_all = np.concatenate([fc1_b, fc2_b, fc3_b[sl]])
        biasT_h = np.zeros((128, 9), dtype=np.float16)
        biasT_h[:, 0:4] = fc1_b.reshape(4, 128).T
        biasT_h[:, 4:8] = fc2_b.reshape(4, 128).T
        biasT_h[0:O3L, 8] = fc3_b[sl]
        nk1b_h = (-k1 * bias_all).astype(np.float16)[None, :]
        in_maps.append(
            dict(xw1a=xw1a_h, w1b=w1b_h, w23=w23_h, biasT=biasT_h, nk1b=nk1b_h)
        )

    res = run_bass_kernel_spmd(nc, in_maps, list(range(N_CORES)))
    global LAST_RESULTS
    LAST_RESULTS = res
    return np.ascontiguousarray(
        np.concatenate(
            [res.results[c]["out"].T.astype(np.float32) for c in range(N_CORES)],
            axis=1,
        )
    )


if __name__ == "__main__":
    rng = np.random.default_rng(0)

    def lin(fo, fi):
        bound = 1.0 / np.sqrt(fi)
        return (
            rng.uniform(-bound, bound, (fo, fi)).astype(np.float32),
            rng.uniform(-bound, bound, (fo,)).astype(np.float32),
        )

    fc1_w, fc1_b = lin(512, 1024)
    fc2_w, fc2_b = lin(512, 512)
    fc3_w, fc3_b = lin(256, 512)
    c1w, c1b = lin(8, 3)
    c2w, c2b = lin(1, 8)
    ins = dict(
        x=rng.standard_normal((32, 1024)).astype(np.float32),
        fc1_w=fc1_w, fc1_b=fc1_b, fc2_w=fc2_w, fc2_b=fc2_b,
        fc3_w=fc3_w, fc3_b=fc3_b,
        conv1_w=c1w, conv1_b=c1b, conv2_w=c2w, conv2_b=c2b,
        batch_num=10,
    )
    out = kernel(**ins)
    print("kernel out", out.shape, out.dtype, float(np.abs(out).max()))


# revision 30
# speedup vs baseline: 1.1012x; 1.1012x over previous
"""Trainium2 Bass kernel for nn_DiffNet (gnn_message_passing).

The reference's per-element "edge MLP" over the meta stack (vi, W, vj)
collapses algebraically.  With g = conv1_w.T @ conv2_w[0] (3 scalars),
hb = conv1_b@conv2_w[0]+conv2_b[0], z = vi @ W.T (no bias),
s1[b] = sum_i vi[b,i], s2[b] = sum_i vi[b,i]^2:

    out[b,o] = relu(z+b)[b,o] * (1 + scale*g2*s1[b])
             + scale*(g0*s2[b] + g1*z[b,o] + hb*s1[b])

so the whole network is 3 matmuls + per-batch stats + elementwise.

Implementation notes (fp16 end-to-end on the DMA/matmul dataflow; the
2e-2 rel-err gate leaves ~10x headroom, measured ~2.6e-3):

  * z is computed TRANSPOSED via weight-stationary matmuls: lhsT =
    [128,128] weight block (fast weight load), rhs = a-chunk [128,B].
    (LDW,MM) pairs pipeline at ~50ns, and zt lands [out-feature, batch]
    in PSUM directly -- no z->SBUF copy, no eye transposes.
  * layer bias accumulates into the same psum group as one rank-1
    matmul (bias_row[1,O] stationary x ones[1,B] moving); the spurious
    k1*bias term this adds to the k1*z part of the combine is ~5e-5
    relative -- far below the gate -- so no correction is applied.
  * per-batch stats: column sums of a and a^2 via matmul chains against
    ones-column stationaries, accumulated chunk-wise in PSUM, so s1
    lands on partition 0 and s2 on partition 32 (where the alpha/beta
    coefficient matmuls need them).
  * alpha/beta: rank-1 matmuls with memset-built [96,128] coefficient
    stationaries -> [*,2B] psum, one small SBUF copy; the wide combine
    reads them through stride-0 broadcast APs:
        m = max(zt,0)*alpha ; t = k1*zt + beta ; a' = m + t  (3 DVE ops)
  * the PE's HAM clock gate defaults to 4/8 throttle (1.2 GHz) and only
    reaches 8/8 after ~4us of sustained matmul activity, so a block of
    junk matmuls warms the array while the weight DMA streams.

Distribution (8 cores, no collectives): fc1/fc2 replicated, fc3 sharded
over its output dim (32 cols/core); full batch B=32 on every core; host
concatenates the 8 [32,32] output shards.
"""

import sys

if "/opt/trn_rl_repo" not in sys.path:
    sys.path.insert(0, "/opt/trn_rl_repo")

import numpy as np


def _install_ntff_hook_shim():
    """This image's antenv lacks ``axon_hooks``; bass_utils hard-imports it
    when tracing under axon.  Provide the module and register the ctypes
    NTFF hook from trn_agent_boot so ``trace=True`` yields exec_time_ns."""
    import types

    if "antenv.axon_hooks" in sys.modules:
        return
    try:
        import antenv

        mod = types.ModuleType("antenv.axon_hooks")
        _h = [None]
        mod.set_axon_ntff_profile_hook = lambda hook: _h.__setitem__(0, hook)
        mod.get_axon_ntff_profile_hook = lambda: _h[0]
        sys.modules["antenv.axon_hooks"] = mod
        antenv.axon_hooks = mod
        from trn_agent_boot.trn_boot import _ntff_profile_via_ctypes

        mod.set_axon_ntff_profile_hook(
            _ntff_profile_via_ctypes("/opt/axon/libaxon_pjrt.so")
        )
    except Exception:
        pass


_install_ntff_hook_shim()

N_CORES = 8
B = 32
I1, O1, O2, O3 = 1024, 512, 512, 256
O3L = O3 // N_CORES  # fc3 output cols per core
RATE = 0.1
N_JUNK = 32  # HAM warmup matmuls (64-col) during the DMA stream

_CACHE = {}
LAST_RESULTS = None  # BassKernelResults of the most recent run (for test.py)


def _build(k0, k1, k2, kb):
    import concourse.bacc as bacc
    import concourse.mybir as mybir
    import concourse.tile as tile
    import concourse.bass as bass

    f16 = mybir.dt.float16
    f32 = mybir.dt.float32
    AF = mybir.ActivationFunctionType
    ALU = mybir.AluOpType

    nc = bacc.Bacc(
        "TRN2", target_bir_lowering=False, debug=False, num_devices=N_CORES
    )

    # DRAM parameters (all fp16).  Weights are packed as [128,128] blocks,
    # block order c-outer / k-inner, so each output chunk's accumulation
    # chain is contiguous in the stream.
    # xw1a: cols 0:256 = x.T packed; 256:2304 = w1 blocks for c=0,1
    xw1a = nc.declare_dram_parameter("xw1a", [128, 2304], f16, isOutput=False)
    w1b = nc.declare_dram_parameter("w1b", [128, 2048], f16, isOutput=False)
    # w23: cols 0:2048 = w2 blocks (c0..c3), 2048:2176 = w3 blocks (k0..k3)
    w23 = nc.declare_dram_parameter("w23", [128, 2176], f16, isOutput=False)
    # biasT: per-layer bias columns for the combine's broadcast add
    # (cols 0:4 = fc1_b chunks, 4:8 = fc2_b, 8 = fc3_b[core] on rows 0:32)
    biasT = nc.declare_dram_parameter("biasT", [128, 9], f16, isOutput=False)
    # nk1b: -k1 * (fc1_b | fc2_b | fc3_b[core]) -> row 64 of the beta
    # stationaries, folding the bias correction into the beta matmuls
    nk1b = nc.declare_dram_parameter("nk1b", [1, O1 + O2 + O3L], f16, isOutput=False)
    out_d = nc.declare_dram_parameter("out", [O3L, B], f16, isOutput=True)

    def rep(ap, n):
        """Insert a stride-0 dim of size n before the innermost free dim."""
        return ap.unsqueeze(1).broadcast_to([ap.shape[0], n, ap.shape[1]])

    with tile.TileContext(nc) as tc:
        with (
            tc.tile_pool(name="sb", bufs=1) as sp,
            tc.tile_pool(name="ps", bufs=1, space=bass.MemorySpace.PSUM) as pp,
        ):
            # ---- SBUF tiles
            txw1a = sp.tile([128, 2304], f16, tag="xw1a")
            tx = txw1a[:, 0:256]
            tw1a = txw1a[:, 256:2304]
            tw1b = sp.tile([128, 2048], f16, tag="w1b")
            tw23 = sp.tile([128, 2176], f16, tag="w23")
            tbiasT = sp.tile([128, 9], f16, tag="biasT")
            txsq = sp.tile([128, 256], f16, tag="xsq")
            tka = sp.tile([65, 128], f16, tag="ka")   # alpha: k2@r0, 1@r64
            # beta stationaries: kb@r0, k0@r32, -k1*bias@r64 (per out col)
            tkb = sp.tile([65, O1 + O2 + O3L], f16, tag="kb")
            tones2 = sp.tile([128, 33], f16, tag="ones2")  # cols 0,32 = 1
            s_sb = [
                sp.tile([65, B], f16, tag=f"ssb{l}", name=f"ssb{l}")
                for l in range(3)
            ]
            tm = [
                sp.tile([128, 4 * B], f16, tag="m1", name="m1"),
                sp.tile([128, 4 * B], f16, tag="m2", name="m2"),
                sp.tile([O3L, B], f16, tag="m3", name="m3"),
            ]
            tt = [
                sp.tile([128, 4 * B], f16, tag="t1", name="t1"),
                sp.tile([128, 4 * B], f16, tag="t2", name="t2"),
                sp.tile([O3L, B], f16, tag="t3", name="t3"),
            ]
            ta2 = sp.tile([128, 4 * B], f16, tag="a2")
            ta2sq = sp.tile([128, 4 * B], f16, tag="a2sq")
            ta3 = sp.tile([128, 4 * B], f16, tag="a3")
            ta3sq = sp.tile([128, 4 * B], f16, tag="a3sq")
            tu = [
                sp.tile([128, 4 * B], f16, tag="u1", name="u1"),
                sp.tile([128, 4 * B], f16, tag="u2", name="u2"),
                sp.tile([O3L, B], f16, tag="u3", name="u3"),
            ]
            out_sb = sp.tile([O3L, B], f16, tag="osb")

            # ---- memsets.  Small ones on gpsimd; the wide tkb rows on the
            # (otherwise idle) DVE so they don't serialize the gpsimd chain.
            g = nc.gpsimd
            g.memset(tka[:], 0.0)
            g.memset(tka[0:1, :], k2)
            g.memset(tka[64:65, :], 1.0)
            g.memset(tones2[:], 0.0)
            g.memset(tones2[:, 0:1], 1.0)
            g.memset(tones2[:, 32:33], 1.0)
            for l in range(3):
                g.memset(s_sb[l][:], 1.0)  # junk rows finite
            nc.vector.memset(tkb[:], 0.0)
            nc.vector.memset(tkb[0:1, :], kb)
            nc.vector.memset(tkb[32:33, :], k0)

            # ---- DMAs.  sync ring: payload in need-order; scalar ring:
            # the tiny bias tensors (nk1b lands in tkb row 64 after the
            # zero memset -- Tile orders the overlap).
            nc.sync.dma_start(txw1a[:], xw1a[:])
            nc.sync.dma_start(tw1b[:], w1b[:])
            nc.sync.dma_start(tw23[:], w23[:])
            nc.scalar.dma_start(tbiasT[:], biasT[:])
            nc.scalar.dma_start(tkb[64:65, :], nk1b[:])

            # PSUM is bank-granular (2KB/partition per tile): pack logical
            # regions into shared bank tiles, grouped by phase.
            # Layout per layer bank: zt [*,C*B] | alpha [*,B] | beta' [*,C*B]
            bankA = pp.tile([128, 512], f32, tag="bkA", name="bankA")
            bankB = pp.tile([128, 512], f32, tag="bkB", name="bankB")
            bankC = pp.tile([O3L, 512], f32, tag="bkC", name="bankC")
            bankS = pp.tile([33, 512], f32, tag="bkS", name="bankS")   # stats
            bankJ = pp.tile([1, 512], f32, tag="bkJ", name="bankJ")    # junk
            zt = [
                bankA[:, 0:4 * B],
                bankB[:, 0:4 * B],
                bankC[:, 0:B],
            ]
            alp = [
                bankA[:, 4 * B:5 * B],
                bankB[:, 4 * B:5 * B],
                bankC[:, B:2 * B],
            ]
            bet = [
                bankA[:, 5 * B:9 * B],
                bankB[:, 5 * B:9 * B],
                bankC[:, 2 * B:3 * B],
            ]
            # merged stats region per layer: row 0 = s1, row 32 = s2
            # (the s2' chain writes [0:33] first, then the s1 chain
            # overwrites row 0) -> a single psum->sbuf cast.
            stp = [bankS[0:33, l * B:(l + 1) * B] for l in range(3)]

            MM = nc.tensor.matmul

            # ---- HAM warmup: junk matmuls release the PE's 4/8 clock
            # throttle (it needs ~2-4us of sustained activity).  Gating the
            # moving operand on the tbiasT DMA (scalar ring, lands a couple
            # of us before the payload) self-times the warmup right before
            # the real z chains; results are never read.
            jmov = rep(tbiasT[0:96, 0:8], 8)  # [96, 8, 8] -> 64 cols
            for _ in range(N_JUNK):
                MM(bankJ[0:1, 0:64], tbiasT[0:96, 0:1], jmov,
                   start=True, stop=True)

            def stats(l, a_t, asq_t, C):
                """s2 = column sums of a^2 -> partition 32 (the [33,B] chain
                also fills partition 0 with junk), then s1 = column sums of
                a overwrites partition 0."""
                for c in range(C):
                    MM(stp[l][0:33, 0:B], tones2[:, 0:33],
                       asq_t[:, c * B:(c + 1) * B],
                       start=(c == 0), stop=(c == C - 1))
                for c in range(C):
                    MM(stp[l][0:1, 0:B], tones2[:, 0:1],
                       a_t[:, c * B:(c + 1) * B],
                       start=(c == 0), stop=(c == C - 1))

            def stats_copies(l):
                # on ACT: keeps the DVE free for the combine chain
                nc.scalar.copy(s_sb[l][0:33, 0:B], stp[l][0:33, 0:B])

            def ab_mms(l, C, ocols, boff):
                """alpha[b] broadcast across partitions; beta'[o,b] =
                kb*s1 + k0*s2 - k1*bias[o] via per-chunk stationaries."""
                mv = s_sb[l][0:65, 0:B]
                MM(alp[l][0:ocols, 0:B], tka[:, 0:ocols], mv,
                   start=True, stop=True)
                for c in range(C):
                    MM(bet[l][0:ocols, c * B:(c + 1) * B],
                       tkb[:, boff + c * ocols:boff + (c + 1) * ocols],
                       mv, start=True, stop=True)

            def zt_chains(l, a_t, w_t, K, C, ocols):
                """zt[l] = (a.T @ w).T (biasless) via weight-stationary
                [128,ocols] blocks, K (LDW,MM) pairs per out-chunk."""
                for c in range(C):
                    dst = zt[l][0:ocols, c * B:(c + 1) * B]
                    for k in range(K):
                        MM(dst, w_t[:, (c * K + k) * ocols:(c * K + k + 1) * ocols],
                           a_t[:, k * B:(k + 1) * B],
                           start=(k == 0), stop=(k == K - 1))

            def bias_bc(l, C, np_out, bcol):
                """biasT column(s) broadcast over batch: [np, C] -> [np, C, B]
                with a stride-0 innermost dim."""
                ap = tbiasT[0:np_out, bcol:bcol + C]
                return ap.unsqueeze(2).broadcast_to([np_out, C, B])

            def combine(l, C, np_out, bcol, a_out, sq_out):
                """u = zt + bias ; a_out = max(u,0)*alpha + (k1*u + beta')
                with alpha/beta' read straight from PSUM (one psum src per
                op) and bias via a stride-0 broadcast of biasT."""
                n = C * B
                ztv = zt[l][0:np_out, 0:n]
                uv = tu[l][0:np_out, 0:n]
                nc.vector.tensor_tensor(uv, ztv, bias_bc(l, C, np_out, bcol),
                                        ALU.add)
                al = alp[l][0:np_out, 0:B]
                if C > 1:
                    al = rep(al, C)
                nc.vector.scalar_tensor_tensor(
                    tm[l][0:np_out, 0:n], uv, 0.0, al, ALU.max, ALU.mult)
                nc.vector.scalar_tensor_tensor(
                    tt[l][0:np_out, 0:n], uv, k1, bet[l][0:np_out, 0:n],
                    ALU.mult, ALU.add)
                nc.vector.tensor_tensor(
                    a_out[0:np_out, 0:n], tm[l][0:np_out, 0:n],
                    tt[l][0:np_out, 0:n], ALU.add)
                if sq_out is not None:
                    nc.vector.tensor_tensor(
                        sq_out[0:np_out, 0:n], a_out[0:np_out, 0:n],
                        a_out[0:np_out, 0:n], ALU.mult)

            # ================= layer 1 =================
            nc.vector.tensor_tensor(txsq[:], tx, tx, ALU.mult)
            stats(0, tx, txsq[:], 8)
            stats_copies(0)
            # c=0,1 blocks stream in xw1a; c=2,3 in w1b
            for c in range(2):
                for k in range(8):
                    MM(zt[0][:, c * B:(c + 1) * B],
                       tw1a[:, (c * 8 + k) * 128:(c * 8 + k + 1) * 128],
                       tx[:, k * B:(k + 1) * B], start=(k == 0), stop=(k == 7))
            for c in range(2):
                for k in range(8):
                    MM(zt[0][:, (2 + c) * B:(3 + c) * B],
                       tw1b[:, (c * 8 + k) * 128:(c * 8 + k + 1) * 128],
                       tx[:, k * B:(k + 1) * B], start=(k == 0), stop=(k == 7))
            ab_mms(0, 4, 128, 0)
            combine(0, 4, 128, 0, ta2[:], ta2sq[:])

            # ================= layer 2 =================
            zt_chains(1, ta2[:], tw23[:, 0:2048], 4, 4, 128)
            stats(1, ta2[:], ta2sq[:], 4)
            stats_copies(1)
            ab_mms(1, 4, 128, O1)
            combine(1, 4, 128, 4, ta3[:], ta3sq[:])

            # ================= layer 3 =================
            zt_chains(2, ta3[:], tw23[:, 2048:2176], 4, 1, O3L)
            stats(2, ta3[:], ta3sq[:], 4)
            stats_copies(2)
            ab_mms(2, 1, O3L, O1 + O2)
            combine(2, 1, O3L, 8, out_sb[:], None)

            nc.sync.dma_start(out_d[:], out_sb[:])

    nc.compile()
    return nc


def kernel(**inputs):
    from concourse.bass_utils import run_bass_kernel_spmd

    x = np.asarray(inputs["x"], dtype=np.float32)
    fc1_w = np.asarray(inputs["fc1_w"], dtype=np.float32)
    fc1_b = np.asarray(inputs["fc1_b"], dtype=np.float32)
    fc2_w = np.asarray(inputs["fc2_w"], dtype=np.float32)
    fc2_b = np.asarray(inputs["fc2_b"], dtype=np.float32)
    fc3_w = np.asarray(inputs["fc3_w"], dtype=np.float32)
    fc3_b = np.asarray(inputs["fc3_b"], dtype=np.float32)
    c1w = np.asarray(inputs["conv1_w"], dtype=np.float32)
    c1b = np.asarray(inputs["conv1_b"], dtype=np.float32)
    c2w = np.asarray(inputs["conv2_w"], dtype=np.float32)
    c2b = np.asarray(inputs["conv2_b"], dtype=np.float32)
    bn = float(np.asarray(inputs["batch_num"]).astype(np.float64))

    scale = np.float32(RATE) / np.float32(bn)
    gv = (c1w.T @ c2w[0]).astype(np.float32)  # [3]
    hb = np.float32(c1b @ c2w[0] + c2b[0])
    k0 = float(scale * gv[0])
    k1 = float(scale * gv[1])
    k2 = float(scale * gv[2])
    kb = float(scale * hb)

    key = (k0, k1, k2, kb)
    if key not in _CACHE:
        _CACHE[key] = _build(*key)
    nc = _CACHE[key]

    def pack_x(m):  # [1024, 32] -> [128, 8*32]
        return np.ascontiguousarray(
            m.reshape(8, 128, B).transpose(1, 0, 2).reshape(128, 8 * B)
        ).astype(np.float16)

    def pack_blocks(wt, K, C, ocols):
        """wt [K*128, C*ocols] -> [128, C*K*ocols], block order c-outer
        k-inner: block (c,k) = wt[k*128:(k+1)*128, c*ocols:(c+1)*ocols]."""
        out = np.empty((128, C * K * ocols), dtype=np.float16)
        for c in range(C):
            for k in range(K):
                out[:, (c * K + k) * ocols:(c * K + k + 1) * ocols] = wt[
                    k * 128:(k + 1) * 128, c * ocols:(c + 1) * ocols
                ]
        return out

    w1_h = pack_blocks(fc1_w.T, 8, 4, 128)  # [128, 4096]
    xw1a_h = np.zeros((128, 2304), dtype=np.float16)
    xw1a_h[:, 0:256] = pack_x(x.T)
    xw1a_h[:, 256:2304] = w1_h[:, 0:2048]
    w1b_h = np.ascontiguousarray(w1_h[:, 2048:4096])
    w2_h = pack_blocks(fc2_w.T, 4, 4, 128)  # [128, 2048]

    in_maps = []
    for c in range(N_CORES):
        sl = slice(c * O3L, (c + 1) * O3L)
        w3_h = pack_blocks(fc3_w[sl].T, 4, 1, O3L)  # [128, 128]
        w23_h = np.concatenate([w2_h, w3_h], axis=1)
        bias_all = np.concatenate([fc1_b, fc2_b, fc3_b[sl]])
        biasT_h = np.zeros((128, 9), dtype=np.float16)
        biasT_h[:, 0:4] = fc1_b.reshape(4, 128).T
        biasT_h[:, 4:8] = fc2_b.reshape(4, 128).T
        biasT_h[0:O3L, 8] = fc3_b[sl]
        nk1b_h = (-k1 * bias_all).astype(np.float16)[None, :]
        in_maps.append(
            dict(xw1a=xw1a_h, w1b=w1b_h, w23=w23_h, biasT=biasT_h, nk1b=nk1b_h)
        )

    res = run_bass_kernel_spmd(nc, in_maps, list(range(N_CORES)))
    global LAST_RESULTS
    LAST_RESULTS = res
    return np.ascontiguousarray(
        np.concatenate(
            [res.results[c]["out"].T.astype(np.float32) for c in range(N_CORES)],
            axis=1,
        )
    )


if __name__ == "__main__":
    rng = np.random.default_rng(0)

    def lin(fo, fi):
        bound = 1.0 / np.sqrt(fi)
        return (
            rng.uniform(-bound, bound, (fo, fi)).astype(np.float32),
            rng.uniform(-bound, bound, (fo,)).astype(np.float32),
        )

    fc1_w, fc1_b = lin(512, 1024)
    fc2_w, fc2_b = lin(512, 512)
    fc3_w, fc3_b = lin(256, 512)
    c1w, c1b = lin(8, 3)
    c2w, c2b = lin(1, 8)
    ins = dict(
        x=rng.standard_normal((32, 1024)).astype(np.float32),
        fc1_w=fc1_w, fc1_b=fc1_b, fc2_w=fc2_w, fc2_b=fc2_b,
        fc3_w=fc3_w, fc3_b=fc3_b,
        conv1_w=c1w, conv1_b=c1b, conv2_w=c2w, conv2_b=c2b,
        batch_num=10,
    )
    out = kernel(**ins)
    print("kernel out", out.shape, out.dtype, float(np.abs(out).max()))
